# revision 32
# baseline (speedup 1.0000x reference)
"""Causal multi-head attention (B=2, S=2048, D=1024, H=16, Dh=64) on 8 TRN2
NeuronCores.

Sharding: core c handles batch c//4 and heads 4*(c%4) .. 4*(c%4)+3 (data
parallel on batch x tensor parallel on heads). Each core is fully
independent: it gets x[b]^T and the 256-wide column slices of Wq/Wk/Wv for
its 4 heads, and returns 16 slabs [65, 512] = unnormalized O^T per
(q-chunk, head): rows 0..63 are sum_k exp(s) * v, row 64 is sum_k exp(s)
(softmax denominator via a ones-column in the V operand). The host divides
and transposes while reassembling the full [2, 2048, 1024] output.

Device kernel (per core); matmul operands bf16 (fp32 PSUM accumulate):
  A. Input DMAs are the first instructions after engine init: Sync queue
     carries wq, wk, wv and x^T chunks 1-3; the Pool-engine DGE queue
     carries the per-k x^T chunk-0 slices concurrently, so the first
     projection matmul can start as soon as wq + one slice land. PE
     warm-up matmuls + exp-table preload run during the transfer.
  B. QT = Wq^T x^T ([c, s] layout, one head pair per 128-partition tile),
     same for KT; V = x Wv in natural [s, c] layout, stored augmented with
     a ones column per head.
  C. Per q chunk (processed in order 0,2,1,3), per head pair, over k tiles
     up to the diagonal: scores^T[k,q] for both heads in one 2-bank PSUM
     tile (row-group packed), Exp on ScalarE (scale=1/8, no max needed),
     causal handling by column pruning + triangular-mask multiplies on
     diagonal blocks (Pool engine, off the exp->PV chain), O^T[d,q] +=
     V_aug^T @ expS in PSUM. Scores for k-tile tt+1 are issued before
     PV(tt) so the PE never waits on the Exp. Projections for later chunks
     are interleaved into the k loops in data-arrival/deadline order so the
     Scalar-paced stretches keep the PE fed. Tail per head: PSUM->SBUF
     copies split h0->DVE / h1->ScalarE, each engine issuing its own
     output DMA so the drains never serialize on one queue.
"""

import ml_dtypes
import numpy as np

import concourse.bass as bass
import concourse.mybir as mybir
import concourse.tile as tile
from concourse.bass_utils import run_bass_kernel_spmd
from concourse.masks import make_upper_triangular

B = 2
S = 2048
D = 1024
H = 16
DH = 64
N_CORES = 8
HPC = 4          # heads per core
CW = HPC * DH    # 256: W column slice width per core
QCH = 512        # q chunk width
ND = D // 128    # 8 d tiles
NS = S // 128    # 16 s tiles
NQ = S // QCH    # 4 q chunks
F32 = mybir.dt.float32
BF16 = mybir.dt.bfloat16
DT = BF16
EXP = mybir.ActivationFunctionType.Exp
MULT = mybir.AluOpType.mult
ADD = mybir.AluOpType.add

_STATE = {}


def _split_sync_waits(nc, max_waits=1):
    """This walrus rejects instructions carrying more than ~2 sem-waits
    ("Too many sync wait commands"). Move excess waits emitted by Tile onto
    same-engine NoOps inserted right before the instruction."""
    n = 0
    for f in nc.m.functions:
        for bb in f.blocks:
            il = bb.instructions
            i = 0
            while i < len(il):
                ins = il[i]
                si = getattr(ins, "sync_info", None)
                if si is not None and len(si.on_wait) > max_waits:
                    waits = list(si.on_wait)
                    keep = waits[len(waits) - max_waits:]
                    extra = waits[: len(waits) - max_waits]
                    ins.sync_info = mybir.SyncInfo(
                        on_wait=keep, on_update=list(si.on_update)
                    )
                    pos = i
                    for j in range(0, len(extra), max_waits):
                        nop = mybir.InstNoOp(
                            name=f"{ins.name}-waitsplit{j}",
                            engine=ins.engine,
                            sync_info=mybir.SyncInfo(
                                on_wait=extra[j : j + max_waits], on_update=[]
                            ),
                            bass_nofuse=True,
                        )
                        il.insert(pos, nop)
                        pos += 1
                        i += 1
                    n += 1
                i += 1
    return n


def _build():
    nc = bass.Bass()
    xt_d = nc.dram_tensor("xt", [NQ, 128, ND, QCH], BF16, kind="ExternalInput")
    wq_d = nc.dram_tensor("wq", [128, 2, ND, 128], BF16, kind="ExternalInput")
    wk_d = nc.dram_tensor("wk", [128, 2, ND, 128], BF16, kind="ExternalInput")
    wv_d = nc.dram_tensor("wv", [128, ND, CW], BF16, kind="ExternalInput")
    bq_d = nc.dram_tensor("bq", [CW], F32, kind="ExternalInput")
    bk_d = nc.dram_tensor("bk", [CW], F32, kind="ExternalInput")
    bv_d = nc.dram_tensor("bv", [CW], BF16, kind="ExternalInput")
    # 8 slabs: [4 q-chunks * 2 head-pair tiles][65 rows][2 heads * 512 q]
    out_d = nc.dram_tensor("out", [2 * NQ, 65, 2 * QCH], BF16, kind="ExternalOutput")

    with tile.TileContext(nc) as tc:
        with (
            tc.tile_pool(name="const", bufs=1) as cp,
            tc.tile_pool(name="big", bufs=1) as bigp,
        ):
            # constants; warm/exp seeds first so PE warm-up and the exp
            # table preload can start as early as possible.
            warm = cp.tile([128, 128], DT, tag="warm")
            nc.gpsimd.memset(warm[:], 1.0)
            exp_seed = cp.tile([128, 1], F32, tag="exps")
            nc.gpsimd.memset(exp_seed[:], 0.0)
            ones32 = cp.tile([128, 128], F32, tag="ones32")
            nc.gpsimd.memset(ones32[:], 1.0)
            tri32 = cp.tile([128, 128], F32, tag="tri32")
            make_upper_triangular(nc, tri32[:], val=1.0, diag=True)
            tri = cp.tile([128, 128], DT, tag="tri")
            nc.vector.tensor_copy(tri[:], tri32[:])

            onesb = cp.tile([1, 128], DT, tag="onesb")
            nc.gpsimd.memset(onesb[:], 1.0)
            ones_d = cp.tile([128, HPC], DT, tag="ones_d")
            nc.gpsimd.memset(ones_d[:], 1.0)

            # exp table preload on ScalarE (before its bias DMA enqueues)
            exp_warm = cp.tile([128, 1], DT, tag="expw")
            nc.scalar.activation(exp_warm[:], exp_seed[:], EXP, scale=0.125)

            # input DMAs on Sync in priority order
            xTall = bigp.tile([128, ND * S], DT, tag="xTall")
            # chunk-major SBUF layout [c][k][512]: chunk DMAs become
            # contiguous 4-8KB runs on BOTH sides
            CB = ND * QCH

            def xts(k, q0, width):
                c, s0 = divmod(q0, QCH)
                base = CB * c + QCH * k + s0
                return xTall[:, base : base + width]
            wqall = bigp.tile([128, ND * CW], DT, tag="wqall")
            wkall = bigp.tile([128, ND * CW], DT, tag="wkall")
            wvall = bigp.tile([128, ND * CW], DT, tag="wvall")

            # input DMAs all on the Sync hardware queue in priority order;
            # biases ride the (slow) Scalar DGE queue off the critical path.
            bqs = cp.tile([128, 2], F32, tag="bqs")
            bks = cp.tile([128, 2], F32, tag="bks")
            bvr = cp.tile([1, CW], DT, tag="bvr")
            nc.scalar.dma_start(out=bqs[:], in_=bq_d.rearrange("(t p) -> p t", p=128))
            nc.scalar.dma_start(out=bks[:], in_=bk_d.rearrange("(t p) -> p t", p=128))
            nc.scalar.dma_start(out=bvr[:], in_=bv_d[None, :])
            nc.sync.dma_start(
                out=wqall[:, 0 : ND * 128],
                in_=wq_d[:, 0].rearrange("p k c -> p (k c)"),
            )
            for k in range(4):  # chunk 0 per-k so the first MMs start early
                nc.sync.dma_start(
                    out=xTall[:, QCH * k : QCH * (k + 1)], in_=xt_d[0, :, k, :]
                )
            nc.sync.dma_start(
                out=wqall[:, ND * 128 :],
                in_=wq_d[:, 1].rearrange("p k c -> p (k c)"),
            )
            for k in range(4, ND):
                nc.sync.dma_start(
                    out=xTall[:, QCH * k : QCH * (k + 1)], in_=xt_d[0, :, k, :]
                )
            nc.sync.dma_start(
                out=wkall[:, 0 : ND * 128],
                in_=wk_d[:, 0].rearrange("p k c -> p (k c)"),
            )
            nc.sync.dma_start(
                out=wkall[:, ND * 128 :],
                in_=wk_d[:, 1].rearrange("p k c -> p (k c)"),
            )
            nc.sync.dma_start(
                out=wvall[:], in_=wv_d[:].rearrange("p k c -> p (k c)")
            )
            for c in range(1, NQ):  # whole chunk, 8KB contiguous per partition
                nc.sync.dma_start(
                    out=xTall[:, CB * c : CB * (c + 1)],
                    in_=xt_d[c, :, :, :].rearrange("p k s -> p (k s)"),
                )

            def wq_sl(k, t):
                return wqall[:, ND * 128 * t + 128 * k : ND * 128 * t + 128 * (k + 1)]

            def wk_sl(k, t):
                return wkall[:, ND * 128 * t + 128 * k : ND * 128 * t + 128 * (k + 1)]

            wv = [wvall[:, CW * k : CW * (k + 1)] for k in range(ND)]

            qt = [bigp.tile([128, S], DT, tag=f"qt{t}", name=f"qt{t}") for t in range(2)]
            kt = [bigp.tile([128, S], DT, tag=f"kt{t}", name=f"kt{t}") for t in range(2)]
            va = [bigp.tile([128, 65 * HPC], DT, tag=f"va{i}", name=f"va{i}") for i in range(NS)]

            def va_slice(tt, hl):
                return va[tt][:, 65 * hl : 65 * hl + 65]

            with (
                tc.tile_pool(name="esp", bufs=24) as esp,
                tc.tile_pool(name="obp", bufs=2) as obp,
                tc.tile_pool(name="pp", bufs=2, space="PSUM") as pp,
                tc.tile_pool(name="psc", bufs=2, space="PSUM") as psc,
                tc.tile_pool(name="pso", bufs=2, space="PSUM") as pso,
            ):
                # PE warm-up: keep the PE busy during the input DMA wait so
                # HAM is at K=8/8 when real matmuls begin.
                wscr = pp.tile([128, 128], F32, tag="ppt", name="warmscr")
                for i in range(44):
                    nc.tensor.matmul(
                        wscr[:], warm[:], warm[:], start=(i == 0), stop=(i == 43)
                    )

                def qk_task(wsl, dstT, bsl, t, j):
                    q0 = QCH * j

                    def run():
                        ppt = pp.tile([128, QCH], F32, tag="ppt", name=f"ppt{t}{j}")
                        for k in range(ND):
                            nc.tensor.matmul(
                                ppt[:],
                                wsl(k, t),
                                xts(k, q0, QCH),
                                start=(k == 0),
                                stop=(k == ND - 1),
                            )
                        nc.vector.tensor_scalar_add(
                            dstT[t][:, q0 : q0 + QCH], ppt[:], bsl[:, t : t + 1]
                        )

                    return run

                def v_task(i):
                    def run():
                        ppv = pp.tile([128, CW], F32, tag="ppt", name=f"ppv{i}")
                        for k in range(ND):
                            nc.tensor.matmul(
                                ppv[:],
                                xts(k, 128 * i, 128),
                                wv[k],
                                start=(k == 0),
                                stop=(k == ND - 1),
                            )
                        nc.vector.tensor_tensor(
                            out=va[i].rearrange("p (h e) -> p h e", h=HPC)[:, :, 0:DH],
                            in0=ppv.rearrange("p (h e) -> p h e", e=DH),
                            in1=bcv.rearrange("p (h e) -> p h e", e=DH),
                            op=ADD,
                        )
                        nc.vector.tensor_copy(
                            va[i].rearrange("p (h e) -> p h e", h=HPC)[:, :, DH : DH + 1],
                            ones_d[:, :, None],
                        )

                    return run

                def ppb_task():
                    ppb = pp.tile([128, CW], F32, tag="ppt", name="ppb")
                    nc.tensor.matmul(ppb[:], onesb[0:1, :], bvr[0:1, :], start=True, stop=True)
                    nc.vector.tensor_copy(bcv[:], ppb[:])

                # Software-pipelined attention: stage s runs scores+exp of
                # head-pair stream s concurrently with the PV matmuls of
                # stream s-1 (their es tiles sit in a deep SBUF ring), so
                # the serial Scalar exp chain never gates a PV and the PE
                # is padded with projection fillers wherever it has slack.
                bcv = cp.tile([128, CW], F32, tag="bcv")
                qk_task(wq_sl, qt, bqs, 0, 0)()
                qk_task(wq_sl, qt, bqs, 1, 0)()  # fills the wk DMA wait
                qk_task(wk_sl, kt, bks, 0, 0)()

                STREAMS = [(0, 0), (0, 1), (2, 0), (2, 1),
                           (3, 0), (3, 1), (1, 0), (1, 1)]

                def nkt(j):
                    return 4 * j + 4

                def scores_step(j, t, tt):
                    q0 = QCH * j
                    r = max(0, 128 * tt - q0)
                    pss = psc.tile([128, 2 * QCH], F32, tag="pss")
                    for h, base in ((0, 0), (1, 64)):
                        nc.tensor.matmul(
                            pss[:, h * QCH + r : (h + 1) * QCH],
                            kt[t][base : base + 64, 128 * tt : 128 * (tt + 1)],
                            qt[t][base : base + 64, q0 + r : q0 + QCH],
                            start=True,
                            stop=True,
                            tile_position=(base, 0),
                        )
                    es = esp.tile([128, 2 * QCH], DT, tag="es")
                    if r == 0:
                        nc.scalar.activation(es[:], pss[:], EXP, scale=0.125)
                    else:
                        nc.scalar.activation(
                            es.rearrange("p (h q) -> p h q", h=2)[:, :, r:QCH],
                            pss.rearrange("p (h q) -> p h q", h=2)[:, :, r:QCH],
                            EXP,
                            scale=0.125,
                        )
                    if tt >= 4 * j:  # diagonal block: triangular mask, both
                        # heads in one DVE op (tri broadcast over h)
                        es3 = es.rearrange("p (h q) -> p h q", h=2)
                        nc.vector.tensor_tensor(
                            out=es3[:, :, r : r + 128],
                            in0=es3[:, :, r : r + 128],
                            in1=tri[:, None, :].broadcast_to([128, 2, 128]),
                            op=MULT,
                        )
                    return es, r

                # filler schedule: stage -> [(due step, task)], deadlines set
                # by qt/kt consumption (scores of a later stage), va
                # consumption (PVs one stage later), and x-chunk DMA arrival.
                fills = {
                    0: [(0, ppb_task), (0, qk_task(wk_sl, kt, bks, 1, 0)),
                        (1, v_task(0)), (2, v_task(1))],
                    1: [(0, v_task(2)), (1, v_task(3)),
                        (2, qk_task(wq_sl, qt, bqs, 0, 2))],
                    2: [(0, qk_task(wk_sl, kt, bks, 0, 1)),
                        (2, qk_task(wk_sl, kt, bks, 0, 2)),
                        (3, v_task(4)), (5, v_task(5)), (7, v_task(6)),
                        (9, v_task(7)),
                        (10, qk_task(wq_sl, qt, bqs, 1, 2)),
                        (11, qk_task(wk_sl, kt, bks, 1, 1))],
                    3: [(0, v_task(8)), (1, v_task(9)), (2, v_task(10)),
                        (3, v_task(11)),
                        (5, qk_task(wk_sl, kt, bks, 1, 2)),
                        (9, qk_task(wq_sl, qt, bqs, 0, 3))],
                    4: [(0, qk_task(wk_sl, kt, bks, 0, 3)),
                        (2, v_task(12)), (4, v_task(13)), (6, v_task(14)),
                        (8, v_task(15)),
                        (10, qk_task(wq_sl, qt, bqs, 1, 3))],
                    5: [(2, qk_task(wk_sl, kt, bks, 1, 3)),
                        (6, qk_task(wq_sl, qt, bqs, 0, 1))],
                    6: [(2, qk_task(wq_sl, qt, bqs, 1, 1))],
                }

                es_bank = {}
                for s in range(len(STREAMS) + 1):
                    cur = STREAMS[s] if s < len(STREAMS) else None
                    prev = STREAMS[s - 1] if s >= 1 else None
                    kc = nkt(cur[0]) if cur else 0
                    kp = nkt(prev[0]) if prev else 0
                    if prev is not None:
                        pj, pt = prev
                        po = [
                            pso.tile([128, QCH], F32, tag="po",
                                     name=f"po{pj}{pt}{h}")
                            for h in range(2)
                        ]
                    stage_fills = list(fills.get(s, []))
                    if cur is not None:
                        last_stream = s == len(STREAMS) - 1
                        if last_stream:
                            # proj pool is idle now: accumulate the final
                            # stream's lo halves there, 2 tiles behind its
                            # own exp chain — fills this stage's Scalar
                            # slack and shrinks the last (Scalar-idle) stage
                            FHQ = 384
                            fq0 = QCH * cur[0]
                            flast_lo = max(
                                tt for tt in range(kc)
                                if max(0, 128 * tt - fq0) < FHQ
                            )
                            po_lo2 = [
                                pp.tile([128, QCH], F32, tag="ppt",
                                        name=f"polo{h}")
                                for h in range(2)
                            ]

                            def lo_pv(tt):
                                r = max(0, 128 * tt - fq0)
                                if r >= FHQ:
                                    return
                                es, _ = es_bank[(s, tt)]
                                for h in range(2):
                                    nc.tensor.matmul(
                                        po_lo2[h][0:65, r:FHQ],
                                        va_slice(tt, 2 * cur[1] + h),
                                        es[:, h * QCH + r : h * QCH + FHQ],
                                        start=(tt == 0),
                                        stop=(tt == flast_lo),
                                    )
                        pv_done = 0
                        for i in range(kc):
                            # emit scores two k-tiles at a time: entering the
                            # 64-row quadrant config after a full-K matmul
                            # costs a ~95ns PE pipeline flush, so halve the
                            # number of entries (psc depth 2 allows exactly
                            # two score tiles in flight)
                            if i % 2 == 0:
                                es_bank[(s, i)] = scores_step(cur[0], cur[1], i)
                                es_bank[(s, i + 1)] = scores_step(
                                    cur[0], cur[1], i + 1
                                )
                            while stage_fills and stage_fills[0][0] <= i:
                                stage_fills.pop(0)[1]()
                            if prev is not None:
                                tgt = min(kp, ((i + 1) * kp + kc - 1) // kc)
                                while pv_done < tgt:
                                    tt = pv_done
                                    es, r = es_bank.pop((s - 1, tt))
                                    for h in range(2):
                                        nc.tensor.matmul(
                                            po[h][0:65, r:QCH],
                                            va_slice(tt, 2 * pt + h),
                                            es[:, h * QCH + r : (h + 1) * QCH],
                                            start=(tt == 0),
                                            stop=(tt == kp - 1),
                                        )
                                    pv_done += 1
                            if last_stream and i >= 2:
                                lo_pv(i - 2)
                        if last_stream:
                            for tt in range(max(0, kc - 2), kc):
                                lo_pv(tt)
                        for _, task in stage_fills:
                            task()
                        if prev is not None:
                            ob = obp.tile([128, 2 * QCH], DT, tag="ob")
                            for h in range(2):
                                nc.vector.tensor_copy(
                                    ob[0:65, QCH * h : QCH * (h + 1)],
                                    po[h][0:65, :],
                                )
                            nc.sync.dma_start(
                                out=out_d[2 * pj + pt, :, :], in_=ob[0:65, :]
                            )
                    else:
                        # final stage: PVs of the last stream, column-split
                        # into separate lo/hi PSUM tiles (psc pool is free
                        # of scores now) so the low halves finish + drain
                        # ~3.4us early with no false PSUM dependency on the
                        # still-accumulating high halves.
                        q0 = QCH * pj
                        HQ = 384  # lo/hi split point inside each 512 q range

                        def rr(tt):
                            return max(0, 128 * tt - q0)

                        obf = obp.tile([128, 2 * QCH], DT, tag="ob")
                        po_hi = psc.tile([128, 2 * QCH], F32, tag="pss")
                        # lo halves were accumulated during the previous
                        # stage (po_lo2, proj pool): drain them while the
                        # high halves accumulate on the PE.
                        for h in range(2):
                            nc.vector.tensor_copy(
                                obf[0:65, QCH * h : QCH * h + HQ],
                                po_lo2[h][0:65, 0:HQ],
                            )
                        nc.sync.dma_start(
                            out=out_d[2 * pj + pt].rearrange(
                                "p (a q) -> p a q", a=2
                            )[:, :, 0:HQ],
                            in_=obf[0:65].rearrange(
                                "p (a q) -> p a q", a=2
                            )[:, :, 0:HQ],
                        )
                        HW_ = QCH - HQ
                        for h in range(2):
                            # head-by-head so h0's hi piece drains while
                            # h1's PVs still run; only one copy+DMA remains
                            # after the very last matmul
                            for tt in range(kp):
                                es, r = (es_bank[(s - 1, tt)] if h == 0
                                         else es_bank.pop((s - 1, tt)))
                                b0 = max(r, HQ)
                                nc.tensor.matmul(
                                    po_hi[0:65, QCH * h + b0 - HQ : QCH * h + QCH - HQ],
                                    va_slice(tt, 2 * pt + h),
                                    es[:, h * QCH + b0 : (h + 1) * QCH],
                                    start=(tt == 0),
                                    stop=(tt == kp - 1),
                                )
                            nc.vector.tensor_copy(
                                obf[0:65, h * QCH + HQ : (h + 1) * QCH],
                                po_hi[0:65, QCH * h : QCH * h + HW_],
                            )
                            nc.sync.dma_start(
                                out=out_d[2 * pj + pt, :, h * QCH + HQ : (h + 1) * QCH],
                                in_=obf[0:65, h * QCH + HQ : (h + 1) * QCH],
                            )

    _split_sync_waits(nc)
    return nc


def _get_nc():
    if "nc" not in _STATE:
        _STATE["nc"] = _build()
    return _STATE["nc"]


def kernel(**inputs):
    x = np.asarray(inputs["x"], dtype=np.float32)
    wq = np.asarray(inputs["Wq"], dtype=np.float32).astype(ml_dtypes.bfloat16)
    wk = np.asarray(inputs["Wk"], dtype=np.float32).astype(ml_dtypes.bfloat16)
    wv = np.asarray(inputs["Wv"], dtype=np.float32).astype(ml_dtypes.bfloat16)
    bq = np.asarray(inputs["bq"], dtype=np.float32)
    bk = np.asarray(inputs["bk"], dtype=np.float32)
    bv = np.asarray(inputs["bv"], dtype=np.float32).astype(ml_dtypes.bfloat16)
    xts = []
    for b in range(B):
        xt = x[b].T.astype(ml_dtypes.bfloat16)          # [1024, 2048]
        xt4 = xt.reshape(ND, 128, NQ, QCH).transpose(2, 1, 0, 3)  # [c][p][k][s]
        xts.append(np.ascontiguousarray(xt4))

    def wprep(w, sl):
        # [1024, 256] -> [128 p][8 k][256 c] so each partition's data is one
        # contiguous 4KB run in DRAM
        return np.ascontiguousarray(w[:, sl].reshape(ND, 128, CW).transpose(1, 0, 2))

    def wprep_t(w, sl):
        # [1024, 256] -> [128 p][2 t][8 k][128 c]: each head-pair half is a
        # contiguous 2KB run per partition, DMA'd separately
        return np.ascontiguousarray(
            w[:, sl].reshape(ND, 128, 2, 128).transpose(1, 2, 0, 3)
        )

    in_maps = []
    for c in range(N_CORES):
        b, hg = divmod(c, HPC)
        sl = slice(CW * hg, CW * (hg + 1))
        in_maps.append(
            {
                "xt": xts[b],
                "wq": wprep_t(wq, sl),
                "wk": wprep_t(wk, sl),
                "wv": wprep(wv, sl),
                "bq": np.ascontiguousarray(bq[sl]),
                "bk": np.ascontiguousarray(bk[sl]),
                "bv": np.ascontiguousarray(bv[sl]),
            }
        )

    nc = _get_nc()
    res = run_bass_kernel_spmd(nc, in_maps, list(range(N_CORES)))
    _STATE["last_result"] = res

    out = np.empty((B, S, D), dtype=np.float32)
    for c in range(N_CORES):
        b, hg = divmod(c, HPC)
        slab = np.asarray(res.results[c]["out"], dtype=np.float32)  # [8,65,1024]
        w = slab[:, :DH, :] / slab[:, DH : DH + 1, :]  # normalize
        # [j*2+t, d, h*512+q] -> [j, q, t, h, d] -> [2048, 256]
        oc = (
            w.reshape(NQ, 2, DH, 2, QCH)
            .transpose(0, 4, 1, 3, 2)
            .reshape(S, CW)
        )
        out[b, :, CW * hg : CW * (hg + 1)] = oc
    return out
